# revision 44
# baseline (speedup 1.0000x reference)
"""Position Attention Module (DANet) on 8 Trainium2 NeuronCores.

Reference computation (per batch b of 4):
  xf = x[b] : [C=512, N=4096]
  q = Wq@xf + bq : [64, N];  k = Wk@xf + bk : [64, N];  v = Wv@xf + bv : [512, N]
  scores[i,j] = q[:,i].k[:,j];  attn = softmax_j(scores)
  out[c,i] = alpha * sum_j v[c,j] attn[i,j]

Sharding: 2 cores per batch, each core owns half the query rows (i), full k/v.
Per-core x is pre-rolled on host so the owned i-half is always columns 0:2048.

Final design, ~217us measured (v1 baseline was 326us; rel-err 3.7e-3
against a 2e-2 budget).  PE busy ~189us, i.e. ~95%% of the dense-PE
floor; the AV matmuls alone are ~109us of irreducible compute.
  - q/k/v projections all single-f32r matmuls (1 cyc/row; v1 ran q/k in
    fp32 at 4 cyc/row), q and k packed into ONE [128, JB] matmul stream
    per block ([WqT|WkT] packed weights); the k half moves to partitions
    0-63 with a sbuf-to-sbuf DMA (compute ops never shift partitions).
  - scoresT [j, i] in ONE f32r matmul per tile (v1 used two): KHL =
    [k_hi; k_lo] K-stack keeps k exact; q participates as q_hi only
    (f32r, 2^-12).  Dropping the k.q_lo term costs ~2e-3 output rel-err
    (numpy-sim validated).
  - scores issued PIPE j-iterations ahead of the AV matmuls so the exp
    (Act) latency hides under the AV group -- v1 stalled ~620ns per
    j-iteration waiting for exp through a 2-deep PSUM rotation.
  - AV: out[c,i] += vT[j,c].T @ exp[j,i] with BOTH operands bf16 (the
    compiler rejects mixed 16/32-bit matmul inputs); same 1 cyc/row but
    halved weight-load bytes, ~5us less PE busy, +1.4e-3 error.
  - softmax denominator split into two accumulation chains (even j-tiles
    on the Pool engine, odd on DVE); ones[128,128] f32r matmuls
    sum+broadcast; DVE reciprocal; fused scaling on eviction.
  - DMA: weights host-pre-tiled to [128, N] row-major and x/out stored
    tile-contiguous so descriptors are full partition rows; tiny consts
    (bv feeds the PE's first instruction) issued before bulk transfers;
    first x blocks spread across three engine DGE queues.
Tried and rejected: fp8 e4m3 3-way-split DoubleRow scores (correct at
3.2e-3 and half the score cycles, but mixing fp8/DoubleRow into the PE
stream disabled weight-load/execute overlap hardware-wide, inflating
every matmul to ~270ns and erasing the gain); SBUF-resident x and a
4-way-split wvt DMA (coarser/competing DMA deps stalled projections);
bf16 DRAM outputs (no gain).  Note: the shared device thermally
throttles under sustained load (PE slices inflate 232ns -> 272ns, ~17%%);
identical NEFFs measured 217-263us across the session.
"""
import numpy as np


def _round_f32r(a):
    """Round fp32 mantissa to 11 bits (f32r / E8M11), round-half-even."""
    m, e = np.frexp(np.asarray(a, np.float32).astype(np.float64))
    return np.ldexp(np.round(np.ldexp(m, 12)), -12).astype(np.float32) * (2.0 ** e).astype(np.float32)


B, C, HW = 4, 512, 4096
CQ = 64
NCORES = 8
IH = HW // 2          # 2048 query rows per core
ITILE = 512           # i-tile (psum free dim)
NITILES = IH // ITILE # 4
JT = 128              # j-tile (contraction chunk for AV / scores lhsT cols)
NJT = HW // JT        # 32
JB = 512              # j-block for projections
NJB = HW // JB        # 8
NCC = C // 128        # 4 contraction chunks of 128 over C
PIPE = 2              # scores run this many j-iterations ahead of AV

_cache = {}


def _build():
    import concourse.bacc as bacc
    import concourse.tile as tile
    import concourse.mybir as mybir
    from concourse.bass_utils import run_bass_kernel_spmd

    f32 = mybir.dt.float32
    f32r = mybir.dt.float32r
    bf16 = mybir.dt.bfloat16
    AF = mybir.ActivationFunctionType

    nc = bacc.Bacc("TRN2", target_bir_lowering=False, debug=False)

    # weights host-pre-tiled to [128, ...] row-major so every DMA descriptor
    # is one full contiguous SBUF partition row (vs 256B strided slices)
    x_d = nc.dram_tensor("x", [NCC, NJB, 128, JB], f32, kind="ExternalInput")
    wqt_d = nc.dram_tensor("wqk", [128, NCC * 128], f32, kind="ExternalInput")
    wvt_d = nc.dram_tensor("wvt", [128, NCC * C], f32, kind="ExternalInput")
    bq_d = nc.dram_tensor("bqk", [128, 1], f32, kind="ExternalInput")
    bv_d = nc.dram_tensor("bv", [1, C], f32, kind="ExternalInput")
    out_d = nc.dram_tensor("out", [NITILES, NCC, 128, ITILE], f32, kind="ExternalOutput")

    with tile.TileContext(nc) as tc:
        with (
            tc.tile_pool(name="const", bufs=1) as cpool,
            tc.tile_pool(name="kq", bufs=1) as kqpool,
            tc.tile_pool(name="vt", bufs=1) as vtpool,
        ):
            # --- constants / weights (single big-row DMAs, sliced in SBUF) ---
            # wqk packs [WqT | WkT] per 128-chunk: one projection matmul
            # stream computes q (rows 0-63) and k (rows 64-127) together
            wqk_t = cpool.tile([128, NCC * 128], f32r, tag="wqkt")
            wvt_t = cpool.tile([128, NCC * C], f32r, tag="wvtt")
            nc.scalar.dma_start(wqk_t[:], wqt_d[:].bitcast(f32r))
            wqk = [wqk_t[:, i * 128:(i + 1) * 128] for i in range(NCC)]
            wvt = [wvt_t[:, i * C:(i + 1) * C] for i in range(NCC)]
            bqk_c = cpool.tile([128, 1], f32, tag="bqkc")
            nc.gpsimd.dma_start(bqk_c[:], bq_d[:])
            # f32r hi/lo split activations for scores (K-stacked):
            #  KHL [128, HW]: rows 0-63 = k_hi, rows 64-127 = k_lo
            #  QHH [128, IH]: q_hi duplicated on both halves
            KHL = kqpool.tile([128, HW], f32r, tag="khl")
            QHH = kqpool.tile([128, IH], f32r, tag="qhh")
            vts = [vtpool.tile([JT, C], bf16, tag=f"vt{j}", name=f"vt{j}") for j in range(NJT)]

            # ---------------- projections ----------------
            with (
                tc.tile_pool(name="xin", bufs=12) as xpool,
                tc.tile_pool(name="evt", bufs=3) as evpool,
                tc.tile_pool(name="pkq", bufs=2, space="PSUM") as pkq,
                tc.tile_pool(name="pvt", bufs=3, space="PSUM") as pvt,
            ):
                # tiny consts first: bv_row feeds the bvB matmul, the PE's
                # first program-order instruction -- it must not queue behind
                # the bulk x transfers
                bv_row = cpool.tile([1, C], f32, tag="bvrow")
                nc.sync.dma_start(bv_row[:], bv_d[:])
                # prefetch the first two jb x-blocks ahead of the bulk consts,
                # spread across engine DGE queues to parallelize issue;
                # jb0 tiles lead so the first projection matmul starts early
                dma_engs = [nc.sync, nc.gpsimd, nc.scalar, nc.sync]
                xts = {}
                for jb in range(2):
                    for cc in range(NCC):
                        jsl = slice(jb * JB, (jb + 1) * JB)
                        t = xpool.tile([128, JB], f32r, tag="x", name=f"x{jb}_{cc}")
                        dma_engs[cc].dma_start(t[:], x_d[cc, jb].bitcast(f32r))
                        xts[(jb, cc)] = t
                    if jb == 0:
                        nc.gpsimd.dma_start(wvt_t[:], wvt_d[:].bitcast(f32r))
                ones_r = cpool.tile([1, 128], f32, tag="onesr")      # K=1 bcast lhsT
                nc.vector.memset(ones_r[:], 1.0)
                ones_sq = cpool.tile([128, 128], f32r, tag="onessq")  # sum+bcast lhsT
                nc.vector.memset(ones_sq[:].bitcast(f32), 1.0)

                # bvB: (alpha*bv) broadcast to 128 partitions (for vT psum evict)
                bvB = cpool.tile([128, C], f32, tag="bvB")
                ps = pvt.tile([128, C], f32, tag="bvps")
                nc.tensor.matmul(ps[:], ones_r[:], bv_row[:], start=True, stop=True)
                nc.vector.tensor_copy(bvB[:], ps[:])

                for jb in range(NJB):
                    jsl = slice(jb * JB, (jb + 1) * JB)
                    xt = []
                    for cc in range(NCC):
                        if (jb, cc) in xts:
                            xt.append(xts.pop((jb, cc)))
                            continue
                        t = xpool.tile([128, JB], f32r, tag="x", name=f"x{jb}_{cc}")
                        nc.sync.dma_start(t[:], x_d[cc, jb].bitcast(f32r))
                        xt.append(t)
                    # packed q/k projection: rows 0-63 = q, rows 64-127 = k
                    kp = pkq.tile([128, JB], f32, tag="kqp")
                    for cc in range(NCC):
                        nc.tensor.matmul(kp[:], wqk[cc], xt[cc],
                                         start=(cc == 0), stop=(cc == NCC - 1))
                    qkf = evpool.tile([128, JB], f32, tag="ev")
                    nc.scalar.activation(qkf[:], kp[:], AF.Identity, bias=bqk_c[:])
                    # k half down to partitions 0-63 via DMA (partition move)
                    ktmp = evpool.tile([CQ, JB], f32, tag="evk")
                    nc.sync.dma_start(ktmp[:], qkf[CQ:128, :])
                    nc.vector.tensor_copy(KHL[0:CQ, jsl], ktmp[:])
                    klo = evpool.tile([CQ, JB], f32r, tag="evlo")
                    nc.vector.tensor_sub(klo[:], ktmp[:], KHL[0:CQ, jsl])
                    nc.sync.dma_start(KHL[CQ:128, jsl], klo[:])
                    if jb < NJB // 2:
                        nc.vector.tensor_copy(QHH[0:CQ, jsl], qkf[0:CQ, :])
                        nc.sync.dma_start(QHH[CQ:128, jsl], QHH[0:CQ, jsl])
                    # vT tiles [128 j, C] in f32r
                    for js in range(JB // JT):
                        vp = pvt.tile([JT, C], f32, tag="vtp")
                        for cc in range(NCC):
                            nc.tensor.matmul(
                                vp[:], xt[cc][:, js * JT:(js + 1) * JT], wvt[cc],
                                start=(cc == 0), stop=(cc == NCC - 1))
                        nc.vector.tensor_add(vts[jb * 4 + js][:], vp[:], bvB[:])

            # ---------------- attention ----------------
            with (
                tc.tile_pool(name="expp", bufs=4) as epool,
                tc.tile_pool(name="dnm", bufs=4) as dpool,
                tc.tile_pool(name="ost", bufs=4) as opool,
                tc.tile_pool(name="rows", bufs=2) as rpool,
                tc.tile_pool(name="pso", bufs=3, space="PSUM") as pso,
                tc.tile_pool(name="pout", bufs=5, space="PSUM") as pout,
            ):
                for it in range(NITILES):
                    isl = slice(it * ITILE, (it + 1) * ITILE)
                    ops = [pout.tile([128, ITILE], f32, tag="op", name=f"op{it}_{i}") for i in range(NCC)]
                    dnmP = dpool.tile([128, ITILE], f32r, tag="dnp")
                    dnmV = dpool.tile([128, ITILE], f32r, tag="dnv")
                    ets = {}
                    for step in range(NJT + PIPE):
                        if step < NJT:
                            j = step
                            jsl = slice(j * JT, (j + 1) * JT)
                            sp = pso.tile([JT, ITILE], f32, tag="sc")
                            nc.tensor.matmul(sp[:], KHL[:, jsl], QHH[:, isl],
                                             start=True, stop=True)
                            et = epool.tile([JT, ITILE], bf16, tag="exp")
                            nc.scalar.activation(et[:], sp[:], AF.Exp)
                            ets[j] = et
                        if step >= PIPE:
                            jd = step - PIPE
                            et = ets.pop(jd)
                            # denominator: even j-tiles chain on Pool, odd on DVE
                            eng, dnm = (nc.gpsimd, dnmP) if jd % 2 == 0 else (nc.vector, dnmV)
                            if jd < 2:
                                eng.tensor_copy(dnm[:], et[:])
                            else:
                                eng.tensor_add(dnm[:], dnm[:], et[:])
                            for cc in range(NCC):
                                nc.tensor.matmul(
                                    ops[cc][:], vts[jd][:, cc * 128:(cc + 1) * 128], et[:],
                                    start=(jd == 0), stop=(jd == NJT - 1))
                    # denomB = column-sums of dnmP+dnmV broadcast to all partitions
                    dB = pso.tile([128, ITILE], f32, tag="sc")
                    nc.tensor.matmul(dB[:], ones_sq[:], dnmP[:], start=True, stop=False)
                    nc.tensor.matmul(dB[:], ones_sq[:], dnmV[:], start=False, stop=True)
                    recipB = rpool.tile([128, ITILE], f32, tag="recipB")
                    nc.vector.reciprocal_approx_fast(out=recipB[:], in_=dB[:])
                    for cc in range(NCC):
                        ot = opool.tile([128, ITILE], f32, tag="ot")
                        nc.vector.tensor_mul(ot[:], ops[cc][:], recipB[:])
                        nc.sync.dma_start(out_d[it, cc], ot[:])

    nc.compile()
    return nc, run_bass_kernel_spmd


def _pretile(w):
    """[C, N] -> [128, NCC*N]: 128-row chunks laid side by side."""
    return np.ascontiguousarray(np.concatenate(
        [w[cc * 128:(cc + 1) * 128, :] for cc in range(NCC)], axis=1))


def _host_inputs(x, Wq, bq, Wk, bk, Wv, bv, alpha):
    x = np.ascontiguousarray(np.asarray(x, dtype=np.float32)).reshape(B, C, HW)
    a = float(np.asarray(alpha, np.float32).reshape(-1)[0])
    wqt_f = np.asarray(Wq, np.float32).T  # [C, CQ]
    wkt_f = np.asarray(Wk, np.float32).T
    # packed per-chunk [WqT | WkT]: [128, NCC*128]
    wqk = _round_f32r(np.ascontiguousarray(np.concatenate(
        [np.concatenate([wqt_f[cc * 128:(cc + 1) * 128, :],
                         wkt_f[cc * 128:(cc + 1) * 128, :]], axis=1)
         for cc in range(NCC)], axis=1)))
    wvt = _round_f32r(_pretile(np.asarray(Wv, np.float32).T * a))
    bqk = np.concatenate([np.asarray(bq, np.float32).reshape(CQ),
                          np.asarray(bk, np.float32).reshape(CQ)]).reshape(128, 1)
    bva = (np.asarray(bv, np.float32) * a).reshape(1, C)

    in_maps = []
    for core in range(NCORES):
        b, ih = core // 2, core % 2
        xb = x[b]
        if ih:
            xb = np.concatenate([xb[:, IH:], xb[:, :IH]], axis=1)
        xr = _round_f32r(np.ascontiguousarray(xb))
        xr = np.ascontiguousarray(
            xr.reshape(NCC, 128, NJB, JB).transpose(0, 2, 1, 3))
        in_maps.append({"x": xr,
                        "wqk": wqk, "wvt": wvt,
                        "bqk": bqk, "bv": bva})
    return in_maps


def kernel(x, Wq, bq, Wk, bk, Wv, bv, alpha, trace=False, trace_kwargs=None):
    if "nc" not in _cache:
        _cache["nc"] = _build()
    nc, run_spmd = _cache["nc"]

    in_maps = _host_inputs(x, Wq, bq, Wk, bk, Wv, bv, alpha)

    kwargs = {}
    if trace:
        kwargs["trace"] = True
        kwargs.update(trace_kwargs or {})
    res = run_spmd(nc, in_maps, list(range(NCORES)), **kwargs)

    out = np.empty((B, C, HW), dtype=np.float32)
    for core in range(NCORES):
        b, ih = core // 2, core % 2
        oc = np.asarray(res.results[core]["out"], dtype=np.float32)
        oc = oc.transpose(1, 2, 0, 3).reshape(C, IH)
        out[b][:, ih * IH:(ih + 1) * IH] = oc
    if trace:
        return out.reshape(B, C, 64, 64), res
    return out.reshape(B, C, 64, 64)
